# revision 14
# baseline (speedup 1.0000x reference)
"""Trainium2 Bass kernel for nn_ContextModel_85993835200994.

PixelCNN-style context model:
  out = round(x)
  masked 5x5 conv (mode-A mask, 12 active taps), 192->384 ch
  h = concat([conv_out, phi])            # 768 ch
  h1 = leaky(h @ w1 + b1)                # 640
  h2 = leaky(h1 @ w2 + b2)               # 640
  cond = h2 @ w3 + b3                    # 384 = [mean(192) | scale(192)]
  lik = Phi((v+.5)/s) - Phi((v-.5)/s),  v = out - mean, s = max(|scale|,.11)

Distribution: data-parallel over batch x image-half -> 8 cores, each
computing a [192, 64, 128] output slice. The conv only looks up/left
(mode A), so each core needs 2 halo rows above its slice only.

Device kernel (per core, fully fused, one pass over 16 row-chunks):
  conv as shifted matmuls (channels on partitions, N=512 = 4 rows x 128
  cols, fp32r = full-rate FP22 matmul). The 2304-wide conv contraction
  (12 taps x 192 ch) is packed into 18 full K=128 matmuls per output
  tile: 12 taps x ch[0:128] plus 6 "paired" matmuls whose partitions
  64-127 hold a shift-adjusted duplicate of ch[128:192], so one K=128
  matmul contracts two taps' 64-channel halves at once. MLP as chained
  matmuls with PSUM->SBUF ACT copies fusing bias+Prelu, and the
  Gaussian-CDF likelihood with fused DVE ops + ACT Erf.

round(x) is computed on the host (it is needed there anyway to build the
zero-padded halo slices); the likelihood is computed on device.
"""

import numpy as np

import concourse.bass as bass
import concourse.mybir as mybir
import concourse.tile as tile
from concourse import bacc
from concourse.bass_utils import run_bass_kernel_spmd

F32 = mybir.dt.float32
F32R = mybir.dt.float32r
AF = mybir.ActivationFunctionType
ALU = mybir.AluOpType

C_LAT = 192
C_PHI = 384
HID = 640
B, H, W = 4, 128, 128
N_CORES = 8
ROWS = 64            # output rows per core
CHUNK = 4            # rows per chunk -> N = 4*128 = 512 matmul free dim
NCH = ROWS // CHUNK  # 16 chunks
# smaller final chunks shrink the end-of-kernel likelihood tail; N=rows*128
# must stay >= 256 (fp32r matmul is full-rate only at free dim >= 256)
CHUNKS = [(i * 4, 4) for i in range(15)] + [(60, 2), (62, 2)]
XR_H = ROWS + 3      # 2 halo rows above + 1 slack row below (pair shift)
XR_W = W + 6         # 2 pad cols each side + 1 slack col (pair shift)
SQRT2 = 1.4142135623730951

# mode-A mask taps as (dy, dx) offsets relative to the output pixel.
# TAPS[j] and TAPS[5+j] (j<5) differ by (+1,0); TAPS[10]/TAPS[11] by (0,+1).
TAPS = [(dy, dx) for dy in (-2, -1) for dx in (-2, -1, 0, 1, 2)] + \
       [(0, -2), (0, -1)]
NT = len(TAPS)  # 12

# knobs for test.py
TRACE = False
LAST_RESULT = None

_CACHE = {}


def _build():
    nc = bacc.Bacc("TRN2", target_bir_lowering=False, debug=False)

    # per-core inputs (f32r tensors carry plain fp32 bits)
    xr_d = nc.dram_tensor("xr", [C_LAT, XR_H, XR_W], F32R, kind="ExternalInput").ap()
    phi_d = nc.dram_tensor("phi", [C_PHI, ROWS, W], F32R, kind="ExternalInput").ap()
    wc0_d = nc.dram_tensor("wc0", [128, NT, C_PHI], F32R, kind="ExternalInput").ap()
    wc1_d = nc.dram_tensor("wc1", [128, 6, C_PHI], F32R, kind="ExternalInput").ap()
    w1_d = nc.dram_tensor("w1", [128, 6, HID], F32R, kind="ExternalInput").ap()
    w2_d = nc.dram_tensor("w2", [128, 5, HID], F32R, kind="ExternalInput").ap()
    w3_d = nc.dram_tensor("w3", [128, 5, 2 * C_LAT], F32R, kind="ExternalInput").ap()
    bc_d = nc.dram_tensor("bc", [128, 3], F32, kind="ExternalInput").ap()
    b1_d = nc.dram_tensor("b1", [128, 5], F32, kind="ExternalInput").ap()
    b2_d = nc.dram_tensor("b2", [128, 5], F32, kind="ExternalInput").ap()
    b3_d = nc.dram_tensor("b3", [128, 4], F32, kind="ExternalInput").ap()
    lik_d = nc.dram_tensor("lik", [C_LAT, ROWS, W], F32, kind="ExternalOutput").ap()

    with tile.TileContext(nc) as tc:
        with tc.tile_pool(name="const", bufs=1) as cpool, \
             tc.tile_pool(name="rp", bufs=2) as rpool, \
             tc.tile_pool(name="pp", bufs=2) as ppool, \
             tc.tile_pool(name="hp", bufs=2) as hpool, \
             tc.tile_pool(name="tp", bufs=8) as tpool, \
             tc.tile_pool(name="ps", bufs=8, space="PSUM") as pspool:

            # const tiles; DMA emission is deferred into chunk 0 in
            # first-use order so startup transfers don't queue ahead of the
            # first chunk's inputs (weights are split per tap / k-tile so
            # the first matmuls start as soon as their slice lands)
            wc0_s = cpool.tile([128, NT, C_PHI], F32R, tag="wc0")
            wc1_s = cpool.tile([128, 6, C_PHI], F32R, tag="wc1")
            w1_s = cpool.tile([128, 6, HID], F32R, tag="w1")
            w2_s = cpool.tile([128, 5, HID], F32R, tag="w2")
            w3_s = cpool.tile([128, 5, 2 * C_LAT], F32R, tag="w3")
            bc_s = cpool.tile([128, 3], F32, tag="bc")
            b1_s = cpool.tile([128, 5], F32, tag="b1")
            b2_s = cpool.tile([128, 5], F32, tag="b2")
            b3_s = cpool.tile([128, 4], F32, tag="b3")

            for ci, (y0, rows) in enumerate(CHUNKS):
                N = rows * 128

                # ch 0-127, rows y0-2 .. y0+rows-1 (padded coords y0 .. y0+rows+1)
                nr = rows + 2
                R0 = rpool.tile([128, nr, W + 4], F32R, tag="R0")
                nc.sync.dma_start(R0[:, 0:3], xr_d[0:128, y0:y0 + 3, 0:W + 4])
                nc.sync.dma_start(R0[:, 3:nr], xr_d[0:128, y0 + 3:y0 + nr, 0:W + 4])
                # pair tiles: parts 0-63 = ch 128-191 unshifted, parts 64-127 =
                # the same channels shifted by (+1,0) [PT1] / (0,+1) [PT2]
                PT1 = rpool.tile([128, nr, W + 4], F32R, tag="PT1")
                nc.sync.dma_start(PT1[0:64], xr_d[128:192, y0:y0 + nr, 0:W + 4])
                nc.sync.dma_start(PT1[64:128], xr_d[128:192, y0 + 1:y0 + nr + 1, 0:W + 4])
                PT2 = rpool.tile([128, nr, W + 4], F32R, tag="PT2")
                nc.sync.dma_start(PT2[0:64], xr_d[128:192, y0:y0 + nr, 0:W + 4])
                nc.sync.dma_start(PT2[64:128], xr_d[128:192, y0:y0 + nr, 1:W + 5])
                if ci == 0:
                    for t in range(NT):
                        nc.sync.dma_start(wc0_s[:, t], wc0_d[:, t])
                    for j in range(6):
                        nc.sync.dma_start(wc1_s[:, j], wc1_d[:, j])
                    nc.sync.dma_start(bc_s[:], bc_d)
                ph = ppool.tile([128, 3, rows, W], F32R, tag="ph")
                phr = phi_d.rearrange("(j p) h w -> p j h w", p=128)
                for j in range(3):
                    nc.sync.dma_start(ph[:, j], phr[:, j, y0:y0 + rows, :])

                # ---- masked conv: 3 m-tiles x 18 K=128 matmuls ----
                # tap-major so chunk-0 compute starts when the first tap lands
                pc = [pspool.tile([128, 512], F32, tag="ps", name=f"pc{m}_{ci}")
                      for m in range(3)]
                for t, (dy, dx) in enumerate(TAPS):
                    rhs = R0[:, 2 + dy:2 + rows + dy, 2 + dx:2 + dx + W]
                    for m in range(3):
                        ms = slice(m * 128, (m + 1) * 128)
                        nc.tensor.matmul(pc[m][:, :N], wc0_s[:, t, ms], rhs,
                                         start=(t == 0), stop=False)
                for j in range(6):
                    if j < 5:
                        dy, dx = TAPS[j]     # (-2, dx); partner (-1, dx)
                        src = PT1
                    else:
                        dy, dx = TAPS[10]    # (0, -2); partner (0, -1)
                        src = PT2
                    rhs = src[:, 2 + dy:2 + rows + dy, 2 + dx:2 + dx + W]
                    for m in range(3):
                        ms = slice(m * 128, (m + 1) * 128)
                        nc.tensor.matmul(pc[m][:, :N], wc1_s[:, j, ms], rhs,
                                         start=False, stop=(j == 5))

                if ci == 0:
                    for k in range(6):
                        nc.sync.dma_start(w1_s[:, k], w1_d[:, k])
                    nc.sync.dma_start(b1_s[:], b1_d)
                # h k-tiles 0..2 = conv + mask_b ; 3..5 = phi
                hcv = hpool.tile([128, 3, N], F32R, tag="hcv")
                for m in range(3):
                    nc.scalar.activation(hcv[:, m], pc[m][:, :N], AF.Identity,
                                         bias=bc_s[:, m:m + 1])

                def hk(k):
                    return hcv[:, k] if k < 3 else ph[:, k - 3]

                # ---- mlp1: 768 -> 640, prelu ----
                p1 = [pspool.tile([128, 512], F32, tag="ps", name=f"p1_{m}_{ci}")
                      for m in range(5)]
                for m in range(5):
                    ms = slice(m * 128, (m + 1) * 128)
                    for k in range(6):
                        nc.tensor.matmul(p1[m][:, :N], w1_s[:, k, ms], hk(k),
                                         start=(k == 0), stop=(k == 5))
                h1 = hpool.tile([128, 5, N], F32R, tag="h1")
                for m in range(5):
                    nc.scalar.activation(h1[:, m], p1[m][:, :N], AF.Prelu,
                                         bias=b1_s[:, m:m + 1], alpha=0.01)

                if ci == 0:
                    for k in range(5):
                        nc.sync.dma_start(w2_s[:, k], w2_d[:, k])
                    nc.sync.dma_start(b2_s[:], b2_d)
                # ---- mlp2: 640 -> 640, prelu ----
                p2 = [pspool.tile([128, 512], F32, tag="ps", name=f"p2_{m}_{ci}")
                      for m in range(5)]
                for m in range(5):
                    ms = slice(m * 128, (m + 1) * 128)
                    for k in range(5):
                        nc.tensor.matmul(p2[m][:, :N], w2_s[:, k, ms], h1[:, k],
                                         start=(k == 0), stop=(k == 4))
                h2 = hpool.tile([128, 5, N], F32R, tag="h2")
                for m in range(5):
                    nc.scalar.activation(h2[:, m], p2[m][:, :N], AF.Prelu,
                                         bias=b2_s[:, m:m + 1], alpha=0.01)

                if ci == 0:
                    for k in range(5):
                        nc.sync.dma_start(w3_s[:, k], w3_d[:, k])
                    nc.sync.dma_start(b3_s[:], b3_d)
                # ---- mlp3: 640 -> 384 = [mean 192 | scale 192] ----
                # m-tiles: [0:128]=meanA [128:192]=meanB [192:320]=scaleA [320:384]=scaleB
                p3 = []
                for mi, (c0, pw) in enumerate(((0, 128), (128, 64), (192, 128), (320, 64))):
                    pt = pspool.tile([pw, 512], F32, tag="ps", name=f"p3_{mi}_{ci}")
                    for k in range(5):
                        nc.tensor.matmul(pt[:, :N], w3_s[:, k, c0:c0 + pw], h2[:, k],
                                         start=(k == 0), stop=(k == 4))
                    p3.append(pt)

                # ---- likelihood, per channel-group ----
                for g, (P, pm, psc, mcol, scol, Rg, ch0) in enumerate((
                        (128, p3[0], p3[2], 0, 2, R0, 0),
                        (64, p3[1], p3[3], 1, 3, PT1, 128))):
                    tg = f"t{g}"
                    Rc = Rg[0:P, 2:2 + rows, 2:2 + W].bitcast(F32)
                    vn = tpool.tile([P, N], F32, tag=tg, name=f"vn{g}_{ci}")
                    nc.vector.scalar_tensor_tensor(
                        vn[:], pm[:, :N], b3_s[0:P, mcol:mcol + 1], Rc,
                        ALU.add, ALU.subtract)
                    # sc = max(|mm*sqrt2 + b3s*sqrt2|, 0.11*sqrt2)  (sqrt2
                    # pre-folded into the scale-half weights on the host)
                    sabs = tpool.tile([P, N], F32, tag=tg, name=f"sa{g}_{ci}")
                    nc.scalar.activation(sabs[:], psc[:, :N], AF.Abs,
                                         bias=b3_s[0:P, scol:scol + 1])
                    sc = tpool.tile([P, N], F32, tag=tg, name=f"sc{g}_{ci}")
                    nc.vector.tensor_scalar_max(sc[:], sabs[:], 0.11 * SQRT2)
                    scr = tpool.tile([P, N], F32, tag=tg, name=f"scr{g}_{ci}")
                    rq = tpool.tile([P, N], F32, tag=tg, name=f"rq{g}_{ci}")
                    nc.vector.reciprocal_approx_accurate(out=rq[:], in_=sc[:],
                                                         scratch=scr[:])
                    ep = tpool.tile([P, N], F32, tag=tg, name=f"ep{g}_{ci}")
                    nc.vector.scalar_tensor_tensor(ep[:], vn[:], -0.5, rq[:],
                                                   ALU.add, ALU.mult)
                    em = tpool.tile([P, N], F32, tag=tg, name=f"em{g}_{ci}")
                    nc.vector.scalar_tensor_tensor(em[:], vn[:], 0.5, rq[:],
                                                   ALU.add, ALU.mult)
                    e1 = tpool.tile([P, N], F32, tag=tg, name=f"e1{g}_{ci}")
                    nc.scalar.activation(e1[:], em[:], AF.Erf)
                    e2 = tpool.tile([P, N], F32, tag=tg, name=f"e2{g}_{ci}")
                    nc.scalar.activation(e2[:], ep[:], AF.Erf)
                    d = tpool.tile([P, N], F32, tag=tg, name=f"d{g}_{ci}")
                    nc.vector.scalar_tensor_tensor(d[:], e2[:], -1.0, e1[:],
                                                   ALU.mult, ALU.add)
                    lk = tpool.tile([P, N], F32, tag=tg, name=f"lk{g}_{ci}")
                    nc.scalar.activation(lk[:], d[:], AF.Copy, scale=0.5)
                    nc.sync.dma_start(lik_d[ch0:ch0 + P, y0:y0 + rows, :], lk[:])

    nc.compile()
    return nc


def _host_pack(mask_w, mask_b, w1, b1, w2, b2, w3, b3):
    wc = np.empty((C_LAT, NT, C_PHI), np.float32)
    for t, (dy, dx) in enumerate(TAPS):
        ky, kx = dy + 2, dx + 2
        wc[:, t, :] = mask_w[:, :, ky, kx].T
    # paired weights for ch 128-191: parts 0-63 = "a" tap, 64-127 = partner
    wc1p = np.empty((128, 6, C_PHI), np.float32)
    for j in range(6):
        ta, tb = (j, 5 + j) if j < 5 else (10, 11)
        wc1p[0:64, j] = wc[128:, ta]
        wc1p[64:128, j] = wc[128:, tb]
    w1p = np.ascontiguousarray(w1.reshape(6, 128, HID).transpose(1, 0, 2))
    w2p = np.ascontiguousarray(w2.reshape(5, 128, HID).transpose(1, 0, 2))
    w3s2 = w3.copy()
    w3s2[:, C_LAT:] *= SQRT2  # fold the 1/sqrt(2) of the CDF into the scale half
    w3p = np.ascontiguousarray(w3s2.reshape(5, 128, 2 * C_LAT).transpose(1, 0, 2))
    bcp = np.ascontiguousarray(mask_b.reshape(3, 128).T)
    b1p = np.ascontiguousarray(b1.reshape(5, 128).T)
    b2p = np.ascontiguousarray(b2.reshape(5, 128).T)
    b3p = np.zeros((128, 4), np.float32)
    b3p[:, 0] = b3[0:128]
    b3p[:64, 1] = b3[128:192]
    b3p[:, 2] = b3[192:320] * SQRT2
    b3p[:64, 3] = b3[320:384] * SQRT2
    return {"wc0": np.ascontiguousarray(wc[:128]), "wc1": wc1p,
            "w1": w1p, "w2": w2p, "w3": w3p,
            "bc": bcp, "b1": b1p, "b2": b2p, "b3": b3p}


def kernel(x, phi, mask_w, mask_b, w1, b1, w2, b2, w3, b3):
    global LAST_RESULT
    x = np.asarray(x, dtype=np.float32)
    phi = np.asarray(phi, dtype=np.float32)
    weights = _host_pack(np.asarray(mask_w, np.float32), np.asarray(mask_b, np.float32),
                         np.asarray(w1, np.float32), np.asarray(b1, np.float32),
                         np.asarray(w2, np.float32), np.asarray(b2, np.float32),
                         np.asarray(w3, np.float32), np.asarray(b3, np.float32))

    R = np.round(x)  # rounds half to even, same as jnp.round

    if "nc" not in _CACHE:
        _CACHE["nc"] = _build()
    nc = _CACHE["nc"]

    in_maps = []
    for c in range(N_CORES):
        b, r0 = c // 2, (c % 2) * ROWS
        # padded rounded slice: rows r0-2 .. r0+64 (67), cols -2 .. 129+2 (134)
        xr_c = np.zeros((C_LAT, XR_H, XR_W), np.float32)
        lo = max(r0 - 2, 0)
        hi = min(r0 + ROWS + 1, H)
        xr_c[:, 2 - (r0 - lo):2 - (r0 - lo) + (hi - lo), 2:2 + W] = R[b, :, lo:hi, :]
        phi_c = np.ascontiguousarray(phi[b, :, r0:r0 + ROWS, :])
        in_maps.append({"xr": xr_c, "phi": phi_c, **weights})

    res = run_bass_kernel_spmd(nc, in_maps, core_ids=list(range(N_CORES)),
                               trace=TRACE)
    LAST_RESULT = res

    lik = np.empty((B, C_LAT, H, W), np.float32)
    for c in range(N_CORES):
        b, r0 = c // 2, (c % 2) * ROWS
        lik[b, :, r0:r0 + ROWS, :] = res.results[c]["lik"]
    return R, lik


# revision 15
# speedup vs baseline: 1.0179x; 1.0179x over previous
"""Trainium2 Bass kernel for nn_ContextModel_85993835200994.

PixelCNN-style context model:
  out = round(x)
  masked 5x5 conv (mode-A mask, 12 active taps), 192->384 ch
  h = concat([conv_out, phi])            # 768 ch
  h1 = leaky(h @ w1 + b1)                # 640
  h2 = leaky(h1 @ w2 + b2)               # 640
  cond = h2 @ w3 + b3                    # 384 = [mean(192) | scale(192)]
  lik = Phi((v+.5)/s) - Phi((v-.5)/s),  v = out - mean, s = max(|scale|,.11)

Distribution: data-parallel over batch x image-half -> 8 cores, each
computing a [192, 64, 128] output slice. The conv only looks up/left
(mode A), so each core needs 2 halo rows above its slice only.

Device kernel (per core, fully fused, one pass over 16 row-chunks):
  conv as shifted matmuls (channels on partitions, N=512 = 4 rows x 128
  cols, fp32r = full-rate FP22 matmul). The 2304-wide conv contraction
  (12 taps x 192 ch) is packed into 18 full K=128 matmuls per output
  tile: 12 taps x ch[0:128] plus 6 "paired" matmuls whose partitions
  64-127 hold a shift-adjusted duplicate of ch[128:192], so one K=128
  matmul contracts two taps' 64-channel halves at once. MLP as chained
  matmuls with PSUM->SBUF ACT copies fusing bias+Prelu, and the
  Gaussian-CDF likelihood with fused DVE ops + ACT Erf.

round(x) is computed on the host (it is needed there anyway to build the
zero-padded halo slices); the likelihood is computed on device.
"""

import numpy as np

import concourse.bass as bass
import concourse.mybir as mybir
import concourse.tile as tile
from concourse import bacc
from concourse.bass_utils import run_bass_kernel_spmd

F32 = mybir.dt.float32
F32R = mybir.dt.float32r
AF = mybir.ActivationFunctionType
ALU = mybir.AluOpType

C_LAT = 192
C_PHI = 384
HID = 640
B, H, W = 4, 128, 128
N_CORES = 8
ROWS = 64            # output rows per core
CHUNK = 4            # rows per chunk -> N = 4*128 = 512 matmul free dim
NCH = ROWS // CHUNK  # 16 chunks
# smaller final chunks shrink the end-of-kernel likelihood tail; N=rows*128
# must stay >= 256 (fp32r matmul is full-rate only at free dim >= 256)
CHUNKS = [(i * 4, 4) for i in range(15)] + [(60, 2), (62, 2)]
XR_H = ROWS + 3      # 2 halo rows above + 1 slack row below (pair shift)
XR_W = W + 6         # 2 pad cols each side + 1 slack col (pair shift)
SQRT2 = 1.4142135623730951

# mode-A mask taps as (dy, dx) offsets relative to the output pixel.
# TAPS[j] and TAPS[5+j] (j<5) differ by (+1,0); TAPS[10]/TAPS[11] by (0,+1).
TAPS = [(dy, dx) for dy in (-2, -1) for dx in (-2, -1, 0, 1, 2)] + \
       [(0, -2), (0, -1)]
NT = len(TAPS)  # 12

# knobs for test.py
TRACE = False
LAST_RESULT = None

_CACHE = {}


def _build():
    nc = bacc.Bacc("TRN2", target_bir_lowering=False, debug=False)

    # per-core inputs (f32r tensors carry plain fp32 bits)
    xr_d = nc.dram_tensor("xr", [C_LAT, XR_H, XR_W], F32R, kind="ExternalInput").ap()
    phi_d = nc.dram_tensor("phi", [C_PHI, ROWS, W], F32R, kind="ExternalInput").ap()
    wc0_d = nc.dram_tensor("wc0", [128, NT, C_PHI], F32R, kind="ExternalInput").ap()
    wc1_d = nc.dram_tensor("wc1", [128, 6, C_PHI], F32R, kind="ExternalInput").ap()
    w1_d = nc.dram_tensor("w1", [128, 6, HID], F32R, kind="ExternalInput").ap()
    w2_d = nc.dram_tensor("w2", [128, 5, HID], F32R, kind="ExternalInput").ap()
    w3_d = nc.dram_tensor("w3", [128, 5, 2 * C_LAT], F32R, kind="ExternalInput").ap()
    bc_d = nc.dram_tensor("bc", [128, 3], F32, kind="ExternalInput").ap()
    b1_d = nc.dram_tensor("b1", [128, 5], F32, kind="ExternalInput").ap()
    b2_d = nc.dram_tensor("b2", [128, 5], F32, kind="ExternalInput").ap()
    b3_d = nc.dram_tensor("b3", [128, 4], F32, kind="ExternalInput").ap()
    lik_d = nc.dram_tensor("lik", [C_LAT, ROWS, W], F32, kind="ExternalOutput").ap()

    with tile.TileContext(nc) as tc:
        with tc.tile_pool(name="const", bufs=1) as cpool, \
             tc.tile_pool(name="rp", bufs=2) as rpool, \
             tc.tile_pool(name="pp", bufs=2) as ppool, \
             tc.tile_pool(name="hp", bufs=2) as hpool, \
             tc.tile_pool(name="tp", bufs=8) as tpool, \
             tc.tile_pool(name="ps", bufs=8, space="PSUM") as pspool:

            # const tiles; DMA emission is deferred into chunk 0 in
            # first-use order so startup transfers don't queue ahead of the
            # first chunk's inputs (weights are split per tap / k-tile so
            # the first matmuls start as soon as their slice lands)
            wc0_s = cpool.tile([128, NT, C_PHI], F32R, tag="wc0")
            wc1_s = cpool.tile([128, 6, C_PHI], F32R, tag="wc1")
            w1_s = cpool.tile([128, 6, HID], F32R, tag="w1")
            w2_s = cpool.tile([128, 5, HID], F32R, tag="w2")
            w3_s = cpool.tile([128, 5, 2 * C_LAT], F32R, tag="w3")
            bc_s = cpool.tile([128, 3], F32, tag="bc")
            b1_s = cpool.tile([128, 5], F32, tag="b1")
            b2_s = cpool.tile([128, 5], F32, tag="b2")
            b3_s = cpool.tile([128, 4], F32, tag="b3")

            for ci, (y0, rows) in enumerate(CHUNKS):
                N = rows * 128

                # ch 0-127, rows y0-2 .. y0+rows-1 (padded coords y0 .. y0+rows+1)
                nr = rows + 2
                R0 = rpool.tile([128, nr, W + 4], F32R, tag="R0")
                nc.sync.dma_start(R0[:, 0:3], xr_d[0:128, y0:y0 + 3, 0:W + 4])
                nc.sync.dma_start(R0[:, 3:nr], xr_d[0:128, y0 + 3:y0 + nr, 0:W + 4])
                # pair tiles: parts 0-63 = ch 128-191 unshifted, parts 64-127 =
                # the same channels shifted by (+1,0) [PT1] / (0,+1) [PT2]
                PT1 = rpool.tile([128, nr, W + 4], F32R, tag="PT1")
                nc.sync.dma_start(PT1[0:64], xr_d[128:192, y0:y0 + nr, 0:W + 4])
                nc.sync.dma_start(PT1[64:128], xr_d[128:192, y0 + 1:y0 + nr + 1, 0:W + 4])
                PT2 = rpool.tile([128, nr, W + 4], F32R, tag="PT2")
                nc.sync.dma_start(PT2[0:64], xr_d[128:192, y0:y0 + nr, 0:W + 4])
                nc.sync.dma_start(PT2[64:128], xr_d[128:192, y0:y0 + nr, 1:W + 5])
                if ci == 0:
                    for t in range(NT):
                        nc.sync.dma_start(wc0_s[:, t], wc0_d[:, t])
                    for j in range(6):
                        nc.sync.dma_start(wc1_s[:, j], wc1_d[:, j])
                    nc.sync.dma_start(bc_s[:], bc_d)
                ph = ppool.tile([128, 3, rows, W], F32R, tag="ph")
                phr = phi_d.rearrange("(j p) h w -> p j h w", p=128)
                for j in range(3):
                    nc.sync.dma_start(ph[:, j], phr[:, j, y0:y0 + rows, :])

                # ---- masked conv: 3 m-tiles x 18 K=128 matmuls ----
                # tap-major so chunk-0 compute starts when the first tap lands
                pc = [pspool.tile([128, 512], F32, tag="ps", name=f"pc{m}_{ci}")
                      for m in range(3)]
                for t, (dy, dx) in enumerate(TAPS):
                    rhs = R0[:, 2 + dy:2 + rows + dy, 2 + dx:2 + dx + W]
                    for m in range(3):
                        ms = slice(m * 128, (m + 1) * 128)
                        nc.tensor.matmul(pc[m][:, :N], wc0_s[:, t, ms], rhs,
                                         start=(t == 0), stop=False)
                for j in range(6):
                    if j < 5:
                        dy, dx = TAPS[j]     # (-2, dx); partner (-1, dx)
                        src = PT1
                    else:
                        dy, dx = TAPS[10]    # (0, -2); partner (0, -1)
                        src = PT2
                    rhs = src[:, 2 + dy:2 + rows + dy, 2 + dx:2 + dx + W]
                    for m in range(3):
                        ms = slice(m * 128, (m + 1) * 128)
                        nc.tensor.matmul(pc[m][:, :N], wc1_s[:, j, ms], rhs,
                                         start=False, stop=(j == 5))

                if ci == 0:
                    for k in range(6):
                        nc.sync.dma_start(w1_s[:, k], w1_d[:, k])
                    nc.sync.dma_start(b1_s[:], b1_d)
                # h k-tiles 0..2 = conv + mask_b ; 3..5 = phi
                hcv = hpool.tile([128, 3, N], F32R, tag="hcv")
                for m in range(3):
                    nc.scalar.activation(hcv[:, m], pc[m][:, :N], AF.Identity,
                                         bias=bc_s[:, m:m + 1])

                def hk(k):
                    return hcv[:, k] if k < 3 else ph[:, k - 3]

                # ---- mlp1: 768 -> 640, prelu ----
                # phi k-tiles (3,4,5) first: they depend only on the phi DMA,
                # so their 15 matmuls can run while the conv result is still
                # being evacuated (fills chunk-boundary PE gaps)
                p1 = [pspool.tile([128, 512], F32, tag="ps", name=f"p1_{m}_{ci}")
                      for m in range(5)]
                for m in range(5):
                    ms = slice(m * 128, (m + 1) * 128)
                    for i, k in enumerate((3, 4, 5, 0, 1, 2)):
                        nc.tensor.matmul(p1[m][:, :N], w1_s[:, k, ms], hk(k),
                                         start=(i == 0), stop=(i == 5))
                h1 = hpool.tile([128, 5, N], F32R, tag="h1")
                for m in range(5):
                    nc.scalar.activation(h1[:, m], p1[m][:, :N], AF.Prelu,
                                         bias=b1_s[:, m:m + 1], alpha=0.01)

                if ci == 0:
                    for k in range(5):
                        nc.sync.dma_start(w2_s[:, k], w2_d[:, k])
                    nc.sync.dma_start(b2_s[:], b2_d)
                # ---- mlp2: 640 -> 640, prelu ----
                p2 = [pspool.tile([128, 512], F32, tag="ps", name=f"p2_{m}_{ci}")
                      for m in range(5)]
                for m in range(5):
                    ms = slice(m * 128, (m + 1) * 128)
                    for k in range(5):
                        nc.tensor.matmul(p2[m][:, :N], w2_s[:, k, ms], h1[:, k],
                                         start=(k == 0), stop=(k == 4))
                h2 = hpool.tile([128, 5, N], F32R, tag="h2")
                for m in range(5):
                    nc.scalar.activation(h2[:, m], p2[m][:, :N], AF.Prelu,
                                         bias=b2_s[:, m:m + 1], alpha=0.01)

                if ci == 0:
                    for k in range(5):
                        nc.sync.dma_start(w3_s[:, k], w3_d[:, k])
                    nc.sync.dma_start(b3_s[:], b3_d)
                # ---- mlp3: 640 -> 384 = [mean 192 | scale 192] ----
                # m-tiles: [0:128]=meanA [128:192]=meanB [192:320]=scaleA [320:384]=scaleB
                p3 = []
                for mi, (c0, pw) in enumerate(((0, 128), (128, 64), (192, 128), (320, 64))):
                    pt = pspool.tile([pw, 512], F32, tag="ps", name=f"p3_{mi}_{ci}")
                    for k in range(5):
                        nc.tensor.matmul(pt[:, :N], w3_s[:, k, c0:c0 + pw], h2[:, k],
                                         start=(k == 0), stop=(k == 4))
                    p3.append(pt)

                # ---- likelihood, per channel-group ----
                for g, (P, pm, psc, mcol, scol, Rg, ch0) in enumerate((
                        (128, p3[0], p3[2], 0, 2, R0, 0),
                        (64, p3[1], p3[3], 1, 3, PT1, 128))):
                    tg = f"t{g}"
                    Rc = Rg[0:P, 2:2 + rows, 2:2 + W].bitcast(F32)
                    vn = tpool.tile([P, N], F32, tag=tg, name=f"vn{g}_{ci}")
                    nc.vector.scalar_tensor_tensor(
                        vn[:], pm[:, :N], b3_s[0:P, mcol:mcol + 1], Rc,
                        ALU.add, ALU.subtract)
                    # sc = max(|mm*sqrt2 + b3s*sqrt2|, 0.11*sqrt2)  (sqrt2
                    # pre-folded into the scale-half weights on the host)
                    sabs = tpool.tile([P, N], F32, tag=tg, name=f"sa{g}_{ci}")
                    nc.scalar.activation(sabs[:], psc[:, :N], AF.Abs,
                                         bias=b3_s[0:P, scol:scol + 1])
                    sc = tpool.tile([P, N], F32, tag=tg, name=f"sc{g}_{ci}")
                    nc.vector.tensor_scalar_max(sc[:], sabs[:], 0.11 * SQRT2)
                    scr = tpool.tile([P, N], F32, tag=tg, name=f"scr{g}_{ci}")
                    rq = tpool.tile([P, N], F32, tag=tg, name=f"rq{g}_{ci}")
                    nc.vector.reciprocal_approx_accurate(out=rq[:], in_=sc[:],
                                                         scratch=scr[:])
                    ep = tpool.tile([P, N], F32, tag=tg, name=f"ep{g}_{ci}")
                    nc.vector.scalar_tensor_tensor(ep[:], vn[:], -0.5, rq[:],
                                                   ALU.add, ALU.mult)
                    em = tpool.tile([P, N], F32, tag=tg, name=f"em{g}_{ci}")
                    nc.vector.scalar_tensor_tensor(em[:], vn[:], 0.5, rq[:],
                                                   ALU.add, ALU.mult)
                    e1 = tpool.tile([P, N], F32, tag=tg, name=f"e1{g}_{ci}")
                    nc.scalar.activation(e1[:], em[:], AF.Erf)
                    e2 = tpool.tile([P, N], F32, tag=tg, name=f"e2{g}_{ci}")
                    nc.scalar.activation(e2[:], ep[:], AF.Erf)
                    d = tpool.tile([P, N], F32, tag=tg, name=f"d{g}_{ci}")
                    nc.vector.scalar_tensor_tensor(d[:], e2[:], -1.0, e1[:],
                                                   ALU.mult, ALU.add)
                    lk = tpool.tile([P, N], F32, tag=tg, name=f"lk{g}_{ci}")
                    nc.scalar.activation(lk[:], d[:], AF.Copy, scale=0.5)
                    nc.sync.dma_start(lik_d[ch0:ch0 + P, y0:y0 + rows, :], lk[:])

    nc.compile()
    return nc


def _host_pack(mask_w, mask_b, w1, b1, w2, b2, w3, b3):
    wc = np.empty((C_LAT, NT, C_PHI), np.float32)
    for t, (dy, dx) in enumerate(TAPS):
        ky, kx = dy + 2, dx + 2
        wc[:, t, :] = mask_w[:, :, ky, kx].T
    # paired weights for ch 128-191: parts 0-63 = "a" tap, 64-127 = partner
    wc1p = np.empty((128, 6, C_PHI), np.float32)
    for j in range(6):
        ta, tb = (j, 5 + j) if j < 5 else (10, 11)
        wc1p[0:64, j] = wc[128:, ta]
        wc1p[64:128, j] = wc[128:, tb]
    w1p = np.ascontiguousarray(w1.reshape(6, 128, HID).transpose(1, 0, 2))
    w2p = np.ascontiguousarray(w2.reshape(5, 128, HID).transpose(1, 0, 2))
    w3s2 = w3.copy()
    w3s2[:, C_LAT:] *= SQRT2  # fold the 1/sqrt(2) of the CDF into the scale half
    w3p = np.ascontiguousarray(w3s2.reshape(5, 128, 2 * C_LAT).transpose(1, 0, 2))
    bcp = np.ascontiguousarray(mask_b.reshape(3, 128).T)
    b1p = np.ascontiguousarray(b1.reshape(5, 128).T)
    b2p = np.ascontiguousarray(b2.reshape(5, 128).T)
    b3p = np.zeros((128, 4), np.float32)
    b3p[:, 0] = b3[0:128]
    b3p[:64, 1] = b3[128:192]
    b3p[:, 2] = b3[192:320] * SQRT2
    b3p[:64, 3] = b3[320:384] * SQRT2
    return {"wc0": np.ascontiguousarray(wc[:128]), "wc1": wc1p,
            "w1": w1p, "w2": w2p, "w3": w3p,
            "bc": bcp, "b1": b1p, "b2": b2p, "b3": b3p}


def kernel(x, phi, mask_w, mask_b, w1, b1, w2, b2, w3, b3):
    global LAST_RESULT
    x = np.asarray(x, dtype=np.float32)
    phi = np.asarray(phi, dtype=np.float32)
    weights = _host_pack(np.asarray(mask_w, np.float32), np.asarray(mask_b, np.float32),
                         np.asarray(w1, np.float32), np.asarray(b1, np.float32),
                         np.asarray(w2, np.float32), np.asarray(b2, np.float32),
                         np.asarray(w3, np.float32), np.asarray(b3, np.float32))

    R = np.round(x)  # rounds half to even, same as jnp.round

    if "nc" not in _CACHE:
        _CACHE["nc"] = _build()
    nc = _CACHE["nc"]

    in_maps = []
    for c in range(N_CORES):
        b, r0 = c // 2, (c % 2) * ROWS
        # padded rounded slice: rows r0-2 .. r0+64 (67), cols -2 .. 129+2 (134)
        xr_c = np.zeros((C_LAT, XR_H, XR_W), np.float32)
        lo = max(r0 - 2, 0)
        hi = min(r0 + ROWS + 1, H)
        xr_c[:, 2 - (r0 - lo):2 - (r0 - lo) + (hi - lo), 2:2 + W] = R[b, :, lo:hi, :]
        phi_c = np.ascontiguousarray(phi[b, :, r0:r0 + ROWS, :])
        in_maps.append({"xr": xr_c, "phi": phi_c, **weights})

    res = run_bass_kernel_spmd(nc, in_maps, core_ids=list(range(N_CORES)),
                               trace=TRACE)
    LAST_RESULT = res

    lik = np.empty((B, C_LAT, H, W), np.float32)
    for c in range(N_CORES):
        b, r0 = c // 2, (c % 2) * ROWS
        lik[b, :, r0:r0 + ROWS, :] = res.results[c]["lik"]
    return R, lik
